# revision 55
# baseline (speedup 1.0000x reference)
"""MultiHeadAttention Trainium2 kernel (8 NeuronCores), v2.

Reference computation (torch-style Linear, x @ W.T):
    k = key @ W_k.T; v = value @ W_v.T; q = query (no projection)
    scores = q @ k.T / sqrt(64) per head; attn = softmax(scores)
    out = (attn @ v) @ W_o.T

Sharding: core = (batch b, head-group g); each core owns 4 heads of one
batch. Projection weights are column-split by head; the final W_o matmul
is a partial sum over the core's 256 head-channels, summed on host.

v2 keys off the cost model's matmul law (time = out_free_size x
cycles_per_row, independent of K/M):
  - attn@V is TRANSPOSED: out[q,65] = exp[t,q-block].T @ v_ext[t,65],
    putting 128 q's (not 65 dims) on PSUM partitions -> 2.05x fewer PE
    cycles than the [65,q] orientation.
  - softmax denominator: column 64 of the transposed accumulator; the
    epilogue is recip + per-partition broadcast mul (DVE, free-dim ops,
    no gpsimd partition_broadcast needed).
  - normalized heads are PE-transposed back ([128q,64] -> [64,128q],
    128 cycles each) and stacked in head-PAIRS so W_o runs with K=128:
    half the W_o matmuls of the K=64 version; bf16 weights.
  - K/V (and W_k/W_v) stream from DRAM as bf16: phase 1 was DMA-bound
    at ~99us for the f32 stream; halving the bytes makes it PE-bound
    (~96us of projections + j0 chase) at 95-97% occupancy.
  - 27 of 192 phase-2 exp chunks run on DVE instead of ScalarE via an
    fp16-bit-space Schraudolph + quadratic mantissa correction
    (max rel err 0.9%, rms 0.26% vs 0.39%/0.17% for the ACT path).
    Their scores route through the W_o PSUM banks and their attnvs are
    emitted 10 iterations late, so neither the score ring nor the
    in-order PE stream ever waits on the 3.4us DVE chain. Offload slots
    sit in W_o-quiet windows (pr1 sweeps t in {8,11,14,17,20,23,26},
    pr0 t in {23,26}); denser placement congests DVE and loses time.

Schedule: phase 1 streams K/V + projections with two chase sweeps (j0,
both head pairs) like v1; j0 epilogues are stage-batched to avoid
cross-engine bubbles in DVE's in-order stream. Phase 2 runs 6 sweeps
software-pipelined (scores(i) -> exp(i-1) -> attnv(i-2)) with ScalarE
measured at 98-100% busy; each sweep's first 8 attnvs are held until
t=8 (acc banks ring through 2 banks; a K=1 zero-matmul re-arms each
bank in 213ns on PE); epilogue and W_o micro-ops are paced 1-2 per
iteration through dedicated half-bank W_o PSUM slots.
"""

import os
import numpy as np

import concourse.bacc as bacc
import concourse.tile as tile
import concourse.mybir as mybir
from concourse.bass_utils import run_bass_kernel_spmd

F32 = mybir.dt.float32
F32R = mybir.dt.float32r
BF16 = mybir.dt.bfloat16
FP16 = mybir.dt.float16
I16 = mybir.dt.int16
EXPF = mybir.ActivationFunctionType.Exp
ALU = mybir.AluOpType

B, NQ, NK, E, H, D = 2, 2048, 4096, 1024, 16, 64
HPC = 4          # heads per core
C = HPC * D      # head-channels per core (256)
TB = 256         # token block for streaming K/V projections
NTB = NK // TB   # 16
TCH = NK // 128  # 32 t-chunks for attention
QT = 512         # q tile
NJ = NQ // QT    # 4

# ---- DVE softmax-exp (fp16-bit-space Schraudolph + quad correction) ----
# exp(s/8) = 2^y, y = s * (log2e/8). i16 = round(y*1024 + 15*1024) gives
# the fp16 bit pattern of 2^k*(1+f) (k=int(y), f=frac(y)); the quadratic
# g(m) ~= 2^(m-1)/m on m in [1,2) corrects the linear mantissa.
EXP_SC = float(0.125 * np.log2(np.e) * 1024.0)
EXP_B0 = float(15.0 * 1024.0)
_mm = (np.arange(1024) / 1024.0 + 1.0).astype(np.float64)
_G = 2.0 ** (_mm - 1) / _mm
_ch = np.polynomial.chebyshev.Chebyshev.fit(_mm, _G, 2)
_p = _ch.convert(kind=np.polynomial.Polynomial)
EXP_A0, EXP_A1, EXP_A2 = [float(v) for v in _p.coef]

_last_results = None
_last_in_maps = None


def _build(dve_exp_every=1):
    """dve_exp_every: in phase 2, every Nth chunk's exp runs on DVE
    (0 = never)."""
    nc = bacc.Bacc("TRN2", target_bir_lowering=False, debug=False, num_devices=8)

    keyT_d = nc.dram_tensor("keyT", [E, NK], BF16, kind="ExternalInput").ap()
    valT_d = nc.dram_tensor("valT", [E, NK], BF16, kind="ExternalInput").ap()
    qT_d = nc.dram_tensor("qT", [C, NQ], F32, kind="ExternalInput").ap()
    wkT_d = nc.dram_tensor("wkT", [E, C], BF16, kind="ExternalInput").ap()
    wvT_d = nc.dram_tensor("wvT", [E, C], BF16, kind="ExternalInput").ap()
    wo2_d = nc.dram_tensor("wo2", [128, 2, E], BF16, kind="ExternalInput").ap()
    eye_d = nc.dram_tensor("eye", [128, 128], BF16, kind="ExternalInput").ap()
    out_d = nc.dram_tensor("out", [NQ, E], F32, kind="ExternalOutput").ap()

    keyT_r = keyT_d.rearrange("(c p) n -> p c n", p=128)
    valT_r = valT_d.rearrange("(c p) n -> p c n", p=128)
    qT_r = qT_d.rearrange("(c p) n -> p c n", p=128).bitcast(F32R)
    wkT_r = wkT_d.rearrange("(c p) n -> p c n", p=128)
    wvT_r = wvT_d.rearrange("(c p) n -> p c n", p=128)

    with tile.TileContext(nc) as tc:
        with (
            tc.tile_pool(name="wpool", bufs=1) as wpool,
            tc.tile_pool(name="stream", bufs=3) as stream,
            tc.tile_pool(name="big", bufs=1) as big,
            tc.tile_pool(name="expp", bufs=14) as expp,
            tc.tile_pool(name="nmp", bufs=12) as nmp,
            tc.tile_pool(name="nmtp", bufs=12) as nmtp,
            tc.tile_pool(name="rcp", bufs=6) as rcp,
            tc.tile_pool(name="outsb", bufs=3) as outsb,
            tc.tile_pool(name="dvexp", bufs=4) as dvexp,
        ):
            # ---- resident weights / q ----
            wk_sb = wpool.tile([128, 8, C], BF16)
            wv_sb = wpool.tile([128, 8, C], BF16)
            wo2_sb = wpool.tile([128, 2, E], BF16)
            eye_sb = wpool.tile([128, 128], BF16)
            q_sb = wpool.tile([128, 2, NQ], F32R)

            # ---- resident kT / v_ext ----
            kT_sb = big.tile([128, 2, NK], F32R)            # [hd%128, hd//128, t]
            vx_sb = big.tile([128, TCH, HPC, D + 1], BF16)  # [t%128, t//128, h, d|1]
            zeros_sb = big.tile([1, QT], BF16)
            nc.vector.memset(zeros_sb[:], 0.0)
            for t in range(TCH):
                nc.gpsimd.memset(vx_sb[:, t, :, D:D + 1], 1.0)

            def emit_zero_acc(acc_tile):
                # zero a whole [128,512] PSUM bank with one K=1 matmul
                # (213ns on PE, arms+clears the full 2KB zero-region)
                nc.tensor.matmul(acc_tile[:], zeros_sb[0:1, 0:128],
                                 zeros_sb[0:1, :], start=True, stop=True,
                                 skip_group_check=True)

            def emit_scores_pair(sdst_a, sdst_b, pr, t, q0):
                nc.tensor.matmul(sdst_a,
                                 kT_sb[0:64, pr, t * 128:(t + 1) * 128],
                                 q_sb[0:64, pr, q0:q0 + QT],
                                 start=True, stop=True, tile_position=(0, 0))
                nc.tensor.matmul(sdst_b,
                                 kT_sb[64:128, pr, t * 128:(t + 1) * 128],
                                 q_sb[64:128, pr, q0:q0 + QT],
                                 start=True, stop=True, tile_position=(64, 0))

            def emit_attnv(accA, accB, ex, pr, t):
                # transposed attn@V: out[q,65] = ex[t, qblk].T @ vx[t, :].
                # The four mc accumulation groups share one PSUM bank, and a
                # start=True zeroes the WHOLE 2KB zero-region — so the acc
                # tile is memset once instead and every matmul accumulates
                # (start=False), which is also order-independent.
                hA, hB = 2 * pr, 2 * pr + 1
                for mc in range(4):
                    nc.tensor.matmul(accA[:, mc * 128:mc * 128 + D + 1],
                                     ex[:, mc * 128:(mc + 1) * 128],
                                     vx_sb[:, t, hA, :],
                                     start=False, stop=(t == TCH - 1),
                                     skip_group_check=True)
                for mc in range(4):
                    nc.tensor.matmul(accB[:, mc * 128:mc * 128 + D + 1],
                                     ex[:, QT + mc * 128:QT + (mc + 1) * 128],
                                     vx_sb[:, t, hB, :],
                                     start=False, stop=(t == TCH - 1),
                                     skip_group_check=True)

            def emit_dve_exp2(ex, s1, s2):
                """exp for a chunk whose scores live in two pwo bank tiles
                (keeps the main score ring off the DVE queue entirely)."""
                i16 = dvexp.tile([128, 2 * QT], I16, tag="i16", bufs=3, name="i16")
                nc.vector.tensor_scalar(i16[:, 0:QT], s1, EXP_SC, EXP_B0,
                                        ALU.mult, ALU.add)
                nc.vector.tensor_scalar(i16[:, QT:2 * QT], s2, EXP_SC, EXP_B0,
                                        ALU.mult, ALU.add)
                _dve_exp_tail(ex, i16)

            def _dve_exp_tail(ex, i16):
                e_lin = i16[:].bitcast(FP16)
                m16 = dvexp.tile([128, 2 * QT], I16, tag="m16", bufs=3, name="m16")
                nc.vector.tensor_scalar(m16[:], i16[:], 0x03FF, 0x3C00,
                                        ALU.bitwise_and, ALU.bitwise_or)
                m = m16[:].bitcast(FP16)
                t1 = dvexp.tile([128, 2 * QT], FP16, tag="t1", bufs=3, name="t1")
                nc.vector.tensor_scalar(t1[:], m, EXP_A2, EXP_A1,
                                        ALU.mult, ALU.add)
                t2 = dvexp.tile([128, 2 * QT], FP16, tag="t2", bufs=3, name="t2")
                nc.vector.tensor_tensor(t2[:], t1[:], m, ALU.mult)
                g = dvexp.tile([128, 2 * QT], FP16, tag="g", bufs=3, name="g")
                nc.vector.tensor_scalar(g[:], t2[:], EXP_A0, None, ALU.add)
                nc.vector.tensor_tensor(ex, e_lin, g[:], ALU.mult)

            def emit_dve_exp(ex, sc):
                """exp(sc*0.125) -> ex (bf16): the PSUM-touching op and the
                final mul on DVE, the middle of the chain on the idle Pool
                engine so DVE stays responsive (~1.8us/chunk instead of
                3.4us)."""
                i16 = dvexp.tile([128, 2 * QT], I16, tag="i16", bufs=3, name="i16")
                nc.vector.tensor_scalar(i16[:], sc, EXP_SC, EXP_B0,
                                        ALU.mult, ALU.add)
                e_lin = i16[:].bitcast(FP16)
                m16 = dvexp.tile([128, 2 * QT], I16, tag="m16", bufs=3, name="m16")
                nc.vector.tensor_scalar(m16[:], i16[:], 0x03FF, 0x3C00,
                                        ALU.bitwise_and, ALU.bitwise_or)
                m = m16[:].bitcast(FP16)
                t1 = dvexp.tile([128, 2 * QT], FP16, tag="t1", bufs=3, name="t1")
                nc.vector.tensor_scalar(t1[:], m, EXP_A2, EXP_A1,
                                        ALU.mult, ALU.add)
                t2 = dvexp.tile([128, 2 * QT], FP16, tag="t2", bufs=3, name="t2")
                nc.vector.tensor_tensor(t2[:], t1[:], m, ALU.mult)
                g = dvexp.tile([128, 2 * QT], FP16, tag="g", bufs=3, name="g")
                nc.vector.tensor_scalar(g[:], t2[:], EXP_A0, None, ALU.add)
                nc.vector.tensor_tensor(ex, e_lin, g[:], ALU.mult)

            def epilogue_unit(acc, nmT_by_mc, half, mc, tail=False):
                """normalize + transpose one (head, mc) block.
                acc: [128,512] psum (4 mc slices of [128,65]). The transpose
                output aliases the just-consumed acc slice (the nm-mul has
                already read it), so no extra PSUM is needed.
                half: 0/1 = which partition-half of nmT gets this head."""
                sl = acc[:, mc * 128:mc * 128 + D + 1]
                rc = rcp.tile([128, 1], F32, tag="rc", name="rc")
                nc.vector.reciprocal(rc[:], sl[:, D:D + 1])
                nm = nmp.tile([128, D], BF16, tag="nm", name="nm")
                nc.vector.tensor_scalar(nm[:], sl[:, 0:D], rc[:], None,
                                        ALU.mult)
                tslot = acc[0:64, mc * 128:mc * 128 + 64].bitcast(BF16)
                nc.tensor.transpose(tslot, nm[:], eye_sb[:])
                dst = nmT_by_mc[mc][64 * half:64 * half + 64, :]
                if tail:
                    nc.scalar.copy(dst, tslot)
                else:
                    nc.vector.tensor_copy(dst, tslot)

            def emit_epilogue(acc, nmT_by_mc, half):
                for mc in range(4):
                    epilogue_unit(acc, nmT_by_mc, half, mc)

            def new_nmT_set():
                return [nmtp.tile([128, 128], BF16, tag="nmt", name="nmt")
                        for _ in range(4)]

            # ============ PHASE 1: stream + projections + j0 chase ============
            # pool creation order fixes banks: chase accs (freed LAST, by j0
            # epilogues) on banks 0-3 where phase-2 accs go; kps/vps + chase
            # score ring (freed at stream end) on banks 4-7 where the phase-2
            # score ring goes.
            nmT_j0 = {0: None, 1: None}   # by pair
            with (
                tc.tile_pool(name="pacc", bufs=4, space="PSUM") as pacc,
                tc.tile_pool(name="pkv", bufs=1, space="PSUM") as pkv,
                tc.tile_pool(name="ps2", bufs=2, space="PSUM") as ps2,
            ):
                o_acc = {}
                for pr in range(2):
                    o_acc[pr] = (pacc.tile([128, QT], F32, tag="acc", name="oA"),
                                 pacc.tile([128, QT], F32, tag="acc", name="oB"))
                    emit_zero_acc(o_acc[pr][0])
                    emit_zero_acc(o_acc[pr][1])

                # critical-path DMA splitting (same as v1)
                kblk0 = stream.tile([128, 8, TB], BF16, tag="kblk", name="kblk0")
                nc.sync.dma_start(wk_sb[:, 0:1, :], wkT_r[:, 0:1, :])
                nc.sync.dma_start(kblk0[:, 0:1, :], keyT_r[:, 0:1, 0:TB])
                nc.sync.dma_start(wk_sb[:, 1:8, :], wkT_r[:, 1:8, :])
                nc.sync.dma_start(kblk0[:, 1:8, :], keyT_r[:, 1:8, 0:TB])
                nc.sync.dma_start(q_sb[:, :, 0:QT], qT_r[:, :, 0:QT])
                nc.sync.dma_start(wv_sb[:], wvT_r)

                def chase_scores(t, pr, q0=0):
                    s1 = ps2.tile([128, QT], F32, tag="ssc", name="s1")
                    s2 = ps2.tile([128, QT], F32, tag="ssc", name="s2")
                    emit_scores_pair(s1[:], s2[:], pr, t, q0)
                    ex = expp.tile([128, 2 * QT], BF16, tag="exp", name="ex")
                    nc.scalar.activation(ex[:, 0:QT], s1[:], EXPF, scale=0.125)
                    nc.scalar.activation(ex[:, QT:2 * QT], s2[:], EXPF, scale=0.125)
                    return ex

                ex_t = {}
                for tb in range(NTB):
                    ts0 = tb * TB
                    if tb == 0:
                        kblk = kblk0
                    else:
                        kblk = stream.tile([128, 8, TB], BF16, tag="kblk", name="kblk")
                        nc.sync.dma_start(kblk[:], keyT_r[:, :, ts0:ts0 + TB])
                    vblk = stream.tile([128, 8, TB], BF16, tag="vblk", name="vblk", bufs=4)
                    nc.sync.dma_start(vblk[:], valT_r[:, :, ts0:ts0 + TB])
                    kps = pkv.tile([128, 2, TB], F32, tag="kps", name="kps")
                    for mc in range(2):
                        for c in range(8):
                            nc.tensor.matmul(kps[:, mc, :], wk_sb[:, c, mc * 128:(mc + 1) * 128],
                                             kblk[:, c, :], start=(c == 0), stop=(c == 7))
                        nc.vector.tensor_copy(kT_sb[:, mc, ts0:ts0 + TB], kps[:, mc, :])
                        if tb > 0:
                            ex_t[(2 * tb - 1, mc)] = chase_scores(2 * tb - 1, mc)
                    vps = pkv.tile([128, 2, C], F32, tag="vps", name="vps")
                    for t2 in range(TB // 128):
                        for c in range(8):
                            nc.tensor.matmul(vps[:, t2, :], vblk[:, c, t2 * 128:(t2 + 1) * 128],
                                             wv_sb[:, c, :], start=(c == 0), stop=(c == 7))
                        tg = tb * (TB // 128) + t2
                        nc.vector.tensor_copy(
                            vx_sb[:, tg, :, 0:D],
                            vps[:, t2, :].rearrange("p (h d) -> p h d", h=HPC))
                        ex_t[(2 * tb, t2)] = chase_scores(2 * tb, t2)
                    for t in (2 * tb - 1, 2 * tb):
                        if t < 0:
                            continue
                        for pr in range(2):
                            emit_attnv(o_acc[pr][0], o_acc[pr][1],
                                       ex_t.pop((t, pr))[:], pr, t)
                # final odd chunk: score tiles borrow the freed projection banks
                tL = NK // 128 - 1
                for mc in range(2):
                    s1 = pkv.tile([128, QT], F32, tag="kps", name="s1t")
                    s2 = pkv.tile([128, QT], F32, tag="vps", name="s2t")
                    emit_scores_pair(s1[:], s2[:], mc, tL, 0)
                    exL = expp.tile([128, 2 * QT], BF16, tag="exp", name="exL")
                    nc.scalar.activation(exL[:, 0:QT], s1[:], EXPF, scale=0.125)
                    nc.scalar.activation(exL[:, QT:2 * QT], s2[:], EXPF, scale=0.125)
                    ex_t[(tL, mc)] = exL
                # phase-2 q tiles + W_o + identity load after the stream
                nc.sync.dma_start(q_sb[:, :, QT:NQ], qT_r[:, :, QT:NQ])
                nc.sync.dma_start(wo2_sb[:], wo2_d)
                nc.sync.dma_start(eye_sb[:], eye_d)
                for pr in range(2):
                    emit_attnv(o_acc[pr][0], o_acc[pr][1],
                               ex_t.pop((tL, pr))[:], pr, tL)
                # j0 epilogues (transposes alias the chase-acc slices).
                # Stage-major emission: all nm-muls back-to-back on DVE, then
                # all transposes on PE, then all copies on the idle ACT —
                # avoids per-unit cross-engine sem bubbles in DVE's in-order
                # stream.
                units = []
                for pr in range(2):
                    nmT_j0[pr] = new_nmT_set()
                    for half in range(2):
                        a_ = o_acc[pr][half]
                        for mc in range(4):
                            units.append((a_, nmT_j0[pr], half, mc))
                nms = []
                for (a_, nmT, half, mc) in units:
                    sl = a_[:, mc * 128:mc * 128 + D + 1]
                    rc = rcp.tile([128, 1], F32, tag="rc", name="rc")
                    nc.vector.reciprocal(rc[:], sl[:, D:D + 1])
                    nm = nmp.tile([128, D], BF16, tag="nm", name="nm")
                    nc.vector.tensor_scalar(nm[:], sl[:, 0:D], rc[:], None,
                                            ALU.mult)
                    nms.append(nm)
                for u, (a_, nmT, half, mc) in enumerate(units):
                    tslot = a_[0:64, mc * 128:mc * 128 + 64].bitcast(BF16)
                    nc.tensor.transpose(tslot, nms[u][:], eye_sb[:])
                for u, (a_, nmT, half, mc) in enumerate(units):
                    tslot = a_[0:64, mc * 128:mc * 128 + 64].bitcast(BF16)
                    nc.vector.tensor_copy(
                        nmT[mc][64 * half:64 * half + 64, :], tslot)

            # ================= PHASE 2: j1..j3 + all W_o =================
            with (
                tc.tile_pool(name="pacc2", bufs=2, space="PSUM") as pacc2,
                tc.tile_pool(name="pwo", bufs=2, space="PSUM") as pwo,
                tc.tile_pool(name="pscore", bufs=2, space="PSUM") as pscore,
            ):
                sweeps = [(j, pr) for j in range(1, NJ) for pr in range(2)]
                NS = len(sweeps)           # 6
                total = NS * TCH           # 192 chunk iterations
                FLUSH_T = 8                # early attnvs emitted at this t
                FLUSH_T0 = 14              # sweep 0 waits out the j0 epilogue

                acc = [None] * NS
                dve_chunks = set()
                deferred = {}
                early = [list() for _ in range(NS)]
                sc_t = [dict() for _ in range(NS)]
                ex_t = [dict() for _ in range(NS)]
                nmT_by_j = {0: nmT_j0, 1: {}, 2: {}, 3: {}}

                filler = []   # (kind, closure); kind: 'ep' cheap, 'wo' heavier

                def wo_micro_ops(j, tail=False):
                    if tail:
                        # tail: halves (N=512) instead of quarters — fewer
                        # cross-engine hops on the drain critical path
                        ops = []
                        for mc in range(4):
                            osb_holder = {}
                            for h2 in range(2):
                                def mkt(mc, h2, osb_holder):
                                    def fn():
                                        psw = pwo.tile([128, QT], F32,
                                                       tag="wps", name="pswt")
                                        for p in range(2):
                                            nc.tensor.matmul(
                                                psw[:],
                                                nmT_by_j[j][p][mc][:],
                                                wo2_sb[:, p,
                                                       h2 * QT:(h2 + 1) * QT],
                                                start=(p == 0), stop=(p == 1))
                                        if h2 == 0:
                                            osb_holder[0] = outsb.tile(
                                                [128, E], F32, tag="osb",
                                                name="osb")
                                        osb = osb_holder[0]
                                        nc.scalar.copy(
                                            osb[:, h2 * QT:(h2 + 1) * QT],
                                            psw[:])
                                        if h2 == 1:
                                            nc.sync.dma_start(
                                                out_d[j * QT + mc * 128:
                                                      j * QT + (mc + 1) * 128,
                                                      :], osb[:])
                                    return fn
                                ops.append(("wo", mkt(mc, h2, osb_holder)))
                        return ops
                    # one unit per (mc, e-quarter): 2 pair-matmuls (N=256,
                    # 107ns each) into a half-bank psw (ring of 2 so the next
                    # unit's matmuls overlap this unit's copy) + osb copy;
                    # DMA the row block after its 4th quarter lands.
                    ops = []
                    for mc in range(4):
                        osb_holder = {}
                        for q4 in range(4):
                            def mk(mc, q4, osb_holder, tail=tail):
                                def fn():
                                    psw = pwo.tile([128, 256], F32, tag="wps",
                                                   name="psw")
                                    for p in range(2):
                                        nc.tensor.matmul(
                                            psw[:],
                                            nmT_by_j[j][p][mc][:],
                                            wo2_sb[:, p, q4 * 256:(q4 + 1) * 256],
                                            start=(p == 0), stop=(p == 1))
                                    if q4 == 0:
                                        osb_holder[0] = outsb.tile(
                                            [128, E], F32, tag="osb", name="osb")
                                    osb = osb_holder[0]
                                    if tail:
                                        nc.scalar.copy(
                                            osb[:, q4 * 256:(q4 + 1) * 256],
                                            psw[:])
                                    else:
                                        nc.vector.tensor_copy(
                                            osb[:, q4 * 256:(q4 + 1) * 256],
                                            psw[:])
                                    if q4 == 3:
                                        nc.sync.dma_start(
                                            out_d[j * QT + mc * 128:
                                                  j * QT + (mc + 1) * 128, :],
                                            osb[:])
                                return fn
                            ops.append(("wo", mk(mc, q4, osb_holder)))
                    return ops

                def sweep_of(i):
                    return i // TCH, i % TCH

                for i in range(total + 2):
                    # ---- stage A: scores(i) ----
                    if i < total:
                        s, t = sweep_of(i)
                        j, pr = sweeps[s]
                        if t == 0:
                            oA = pacc2.tile([128, QT], F32, tag="acc", name="oA")
                            oB = pacc2.tile([128, QT], F32, tag="acc", name="oB")
                            acc[s] = (oA, oB)
                        if dve_exp_every and ((pr == 1 and t in (8, 11, 14, 17, 20, 23, 26)) or (pr == 0 and t in (23, 26))):
                            sd1 = pwo.tile([128, QT], F32, tag="wps", name="sd1")
                            sd2 = pwo.tile([128, QT], F32, tag="wps", name="sd2")
                            emit_scores_pair(sd1[:], sd2[:], pr, t, j * QT)
                            sc_t[s][t] = (sd1, sd2)
                        else:
                            sc = pscore.tile([128, 2 * QT], F32, tag="sc",
                                             name="sc")
                            emit_scores_pair(sc[:, 0:QT], sc[:, QT:2 * QT], pr,
                                             t, j * QT)
                            sc_t[s][t] = sc
                    # ---- stage B: exp(i-1) ----
                    if 1 <= i <= total:
                        s, t = sweep_of(i - 1)
                        sc = sc_t[s].pop(t)
                        ex = expp.tile([128, 2 * QT], BF16, tag="exp", name="ex")
                        if isinstance(sc, tuple):
                            emit_dve_exp2(ex[:], sc[0][:], sc[1][:])
                            dve_chunks.add((s, t))
                        else:
                            nc.scalar.activation(ex[:], sc[:], EXPF, scale=0.125)
                        ex_t[s][t] = ex
                    # ---- deferred attnv of DVE-exp chunks (the PE stream is
                    # in-order: emitting these at i-2 would stall PE ~2.3us
                    # behind the 3.4us DVE chain; accumulation order is free
                    # thanks to zeroing+start=False, so emit 4 iters late) ----
                    if i in deferred:
                        for (s2, t2, ex2) in deferred.pop(i):
                            j2, pr2 = sweeps[s2]
                            emit_attnv(acc[s2][0], acc[s2][1], ex2[:], pr2, t2)
                    # ---- stage C: attnv(i-2) ----
                    if i >= 2:
                        s, t = sweep_of(i - 2)
                        j, pr = sweeps[s]
                        ex = ex_t[s].pop(t)
                        ft = FLUSH_T0 if s == 0 else FLUSH_T
                        if t < ft:
                            # held until the previous sweep's paced epilogue
                            # has consumed the acc banks this sweep reuses
                            early[s].append((t, ex))
                        elif t == ft:
                            emit_zero_acc(acc[s][0])
                            emit_zero_acc(acc[s][1])
                            for (t2, ex2) in early[s]:
                                emit_attnv(acc[s][0], acc[s][1], ex2[:], pr, t2)
                            early[s] = []
                            emit_attnv(acc[s][0], acc[s][1], ex[:], pr, t)
                        elif (s, t) in dve_chunks:
                            deferred.setdefault(i + 10, []).append((s, t, ex))
                        else:
                            emit_attnv(acc[s][0], acc[s][1], ex[:], pr, t)
                        if t == TCH - 1:
                            # flush any still-deferred attnvs of this sweep
                            # before its epilogue reads the accumulators
                            for rel in sorted(deferred):
                                keep = []
                                for (s2, t2, ex2) in deferred[rel]:
                                    if s2 == s:
                                        emit_attnv(acc[s][0], acc[s][1],
                                                   ex2[:], pr, t2)
                                    else:
                                        keep.append((s2, t2, ex2))
                                if keep:
                                    deferred[rel] = keep
                                else:
                                    del deferred[rel]
                            if pr == 0:
                                nmT_by_j[j] = {0: None, 1: None}
                            nmT_by_j[j][pr] = new_nmT_set()
                            accA, accB = acc[s]
                            nmT = nmT_by_j[j][pr]
                            for mc_ in range(4):
                                for half_, a_ in ((0, accA), (1, accB)):
                                    tail_ = False
                                    def mk_ep(a_, half_, mc_, nmT=nmT,
                                              tail_=tail_):
                                        def fn():
                                            epilogue_unit(a_, nmT, half_, mc_,
                                                          tail=tail_)
                                        return fn
                                    filler.append(("ep", mk_ep(a_, half_, mc_)))
                            acc[s] = None
                            if pr == 1:
                                filler.extend(wo_micro_ops(j, tail=(s == NS - 1)))
                    if i == 4:
                        filler.extend(wo_micro_ops(0))
                    # ---- stage D: paced epilogue/W_o micro-ops: epilogues
                    # 2/iteration; W_o units only on even iterations so their
                    # psw/copy chain never bunches against the DVE queue ----
                    if filler:
                        if filler[0][0] == "ep":
                            filler.pop(0)[1]()
                            if filler and filler[0][0] == "ep":
                                filler.pop(0)[1]()
                        elif i % 2 == 0:
                            filler.pop(0)[1]()

                for kind, fn in filler:
                    fn()

    nc.compile()
    return nc


_nc = None


def kernel(query, key, value, W_k, W_v, W_o):
    global _nc, _last_results, _last_in_maps
    if _nc is None:
        _nc = _build()

    import ml_dtypes
    query = np.asarray(query, dtype=np.float32)
    key = np.asarray(key, dtype=np.float32)
    value = np.asarray(value, dtype=np.float32)
    W_k = np.asarray(W_k, dtype=np.float32)
    W_v = np.asarray(W_v, dtype=np.float32)
    W_o = np.asarray(W_o, dtype=np.float32)

    keyT = [np.ascontiguousarray(key[b].T).astype(ml_dtypes.bfloat16)
            for b in range(B)]
    valT = [np.ascontiguousarray(value[b].T).astype(ml_dtypes.bfloat16)
            for b in range(B)]
    eye = np.eye(128, dtype=np.float32).astype(ml_dtypes.bfloat16)

    in_maps = []
    for b in range(B):
        for g in range(4):
            c0 = g * C
            wo2 = np.ascontiguousarray(
                W_o[:, c0:c0 + C].T.reshape(2, 128, E).transpose(1, 0, 2)
            ).astype(ml_dtypes.bfloat16)
            in_maps.append({
                "keyT": keyT[b],
                "valT": valT[b],
                "qT": np.ascontiguousarray(query[b][:, c0:c0 + C].T),
                "wkT": np.ascontiguousarray(
                    W_k[c0:c0 + C, :].T).astype(ml_dtypes.bfloat16),
                "wvT": np.ascontiguousarray(
                    W_v[c0:c0 + C, :].T).astype(ml_dtypes.bfloat16),
                "wo2": wo2,
                "eye": eye,
            })

    _last_in_maps = in_maps
    res = run_bass_kernel_spmd(
        _nc, in_maps, core_ids=list(range(8)),
        trace=bool(os.environ.get("BASS_TRACE")))
    _last_results = res

    out = np.zeros((B, NQ, E), dtype=np.float32)
    for b in range(B):
        for g in range(4):
            out[b] += res.results[b * 4 + g]["out"]
    return out


# revision 56
# speedup vs baseline: 1.0034x; 1.0034x over previous
"""MultiHeadAttention Trainium2 kernel (8 NeuronCores), v2.

Reference computation (torch-style Linear, x @ W.T):
    k = key @ W_k.T; v = value @ W_v.T; q = query (no projection)
    scores = q @ k.T / sqrt(64) per head; attn = softmax(scores)
    out = (attn @ v) @ W_o.T

Sharding: core = (batch b, head-group g); each core owns 4 heads of one
batch. Projection weights are column-split by head; the final W_o matmul
is a partial sum over the core's 256 head-channels, summed on host.

v2 keys off the cost model's matmul law (time = out_free_size x
cycles_per_row, independent of K/M):
  - attn@V is TRANSPOSED: out[q,65] = exp[t,q-block].T @ v_ext[t,65],
    putting 128 q's (not 65 dims) on PSUM partitions -> 2.05x fewer PE
    cycles than the [65,q] orientation.
  - softmax denominator: column 64 of the transposed accumulator; the
    epilogue is recip + per-partition broadcast mul (DVE, free-dim ops,
    no gpsimd partition_broadcast needed).
  - normalized heads are PE-transposed back ([128q,64] -> [64,128q],
    128 cycles each) and stacked in head-PAIRS so W_o runs with K=128:
    half the W_o matmuls of the K=64 version; bf16 weights.
  - K/V (and W_k/W_v) stream from DRAM as bf16: phase 1 was DMA-bound
    at ~99us for the f32 stream; halving the bytes makes it PE-bound
    (~96us of projections + j0 chase) at 95-97% occupancy.
  - 27 of 192 phase-2 exp chunks run on DVE instead of ScalarE via an
    fp16-bit-space Schraudolph + quadratic mantissa correction
    (max rel err 0.9%, rms 0.26% vs 0.39%/0.17% for the ACT path).
    Their scores route through the W_o PSUM banks and their attnvs are
    emitted 10 iterations late, so neither the score ring nor the
    in-order PE stream ever waits on the 3.4us DVE chain. Offload slots
    sit in W_o-quiet windows (pr1 sweeps t in {8,11,14,17,20,23,26},
    pr0 t in {23,26}); denser placement congests DVE and loses time.

Schedule: phase 1 streams K/V + projections with two chase sweeps (j0,
both head pairs) like v1; j0 epilogues are stage-batched to avoid
cross-engine bubbles in DVE's in-order stream. Phase 2 runs 6 sweeps
software-pipelined (scores(i) -> exp(i-1) -> attnv(i-2)) with ScalarE
measured at 98-100% busy; each sweep's first 8 attnvs are held until
t=8 (acc banks ring through 2 banks; a K=1 zero-matmul re-arms each
bank in 213ns on PE); epilogue and W_o micro-ops are paced 1-2 per
iteration through dedicated half-bank W_o PSUM slots.
"""

import os
import numpy as np

import concourse.bacc as bacc
import concourse.tile as tile
import concourse.mybir as mybir
from concourse.bass_utils import run_bass_kernel_spmd

F32 = mybir.dt.float32
F32R = mybir.dt.float32r
BF16 = mybir.dt.bfloat16
FP16 = mybir.dt.float16
I16 = mybir.dt.int16
EXPF = mybir.ActivationFunctionType.Exp
ALU = mybir.AluOpType

B, NQ, NK, E, H, D = 2, 2048, 4096, 1024, 16, 64
HPC = 4          # heads per core
C = HPC * D      # head-channels per core (256)
TB = 256         # token block for streaming K/V projections
NTB = NK // TB   # 16
TCH = NK // 128  # 32 t-chunks for attention
QT = 512         # q tile
NJ = NQ // QT    # 4

# ---- DVE softmax-exp (fp16-bit-space Schraudolph + quad correction) ----
# exp(s/8) = 2^y, y = s * (log2e/8). i16 = round(y*1024 + 15*1024) gives
# the fp16 bit pattern of 2^k*(1+f) (k=int(y), f=frac(y)); the quadratic
# g(m) ~= 2^(m-1)/m on m in [1,2) corrects the linear mantissa.
EXP_SC = float(0.125 * np.log2(np.e) * 1024.0)
EXP_B0 = float(15.0 * 1024.0)
_mm = (np.arange(1024) / 1024.0 + 1.0).astype(np.float64)
_G = 2.0 ** (_mm - 1) / _mm
_ch = np.polynomial.chebyshev.Chebyshev.fit(_mm, _G, 2)
_p = _ch.convert(kind=np.polynomial.Polynomial)
EXP_A0, EXP_A1, EXP_A2 = [float(v) for v in _p.coef]

_last_results = None
_last_in_maps = None


def _build(dve_exp_every=1):
    """dve_exp_every: in phase 2, every Nth chunk's exp runs on DVE
    (0 = never)."""
    nc = bacc.Bacc("TRN2", target_bir_lowering=False, debug=False, num_devices=8)

    keyT_d = nc.dram_tensor("keyT", [E, NK], BF16, kind="ExternalInput").ap()
    valT_d = nc.dram_tensor("valT", [E, NK], BF16, kind="ExternalInput").ap()
    qT_d = nc.dram_tensor("qT", [C, NQ], F32, kind="ExternalInput").ap()
    wkT_d = nc.dram_tensor("wkT", [E, C], BF16, kind="ExternalInput").ap()
    wvT_d = nc.dram_tensor("wvT", [E, C], BF16, kind="ExternalInput").ap()
    wo2_d = nc.dram_tensor("wo2", [128, 2, E], BF16, kind="ExternalInput").ap()
    eye_d = nc.dram_tensor("eye", [128, 128], BF16, kind="ExternalInput").ap()
    out_d = nc.dram_tensor("out", [NQ, E], F32, kind="ExternalOutput").ap()

    keyT_r = keyT_d.rearrange("(c p) n -> p c n", p=128)
    valT_r = valT_d.rearrange("(c p) n -> p c n", p=128)
    qT_r = qT_d.rearrange("(c p) n -> p c n", p=128).bitcast(F32R)
    wkT_r = wkT_d.rearrange("(c p) n -> p c n", p=128)
    wvT_r = wvT_d.rearrange("(c p) n -> p c n", p=128)

    with tile.TileContext(nc) as tc:
        with (
            tc.tile_pool(name="wpool", bufs=1) as wpool,
            tc.tile_pool(name="stream", bufs=3) as stream,
            tc.tile_pool(name="big", bufs=1) as big,
            tc.tile_pool(name="expp", bufs=14) as expp,
            tc.tile_pool(name="nmp", bufs=12) as nmp,
            tc.tile_pool(name="nmtp", bufs=12) as nmtp,
            tc.tile_pool(name="rcp", bufs=6) as rcp,
            tc.tile_pool(name="outsb", bufs=3) as outsb,
            tc.tile_pool(name="dvexp", bufs=4) as dvexp,
        ):
            # ---- resident weights / q ----
            wk_sb = wpool.tile([128, 8, C], BF16)
            wv_sb = wpool.tile([128, 8, C], BF16)
            wo2_sb = wpool.tile([128, 2, E], BF16)
            eye_sb = wpool.tile([128, 128], BF16)
            q_sb = wpool.tile([128, 2, NQ], F32R)

            # ---- resident kT / v_ext ----
            kT_sb = big.tile([128, 2, NK], F32R)            # [hd%128, hd//128, t]
            vx_sb = big.tile([128, TCH, HPC, D + 1], BF16)  # [t%128, t//128, h, d|1]
            zeros_sb = big.tile([1, QT], BF16)
            nc.vector.memset(zeros_sb[:], 0.0)
            for t in range(TCH):
                nc.gpsimd.memset(vx_sb[:, t, :, D:D + 1], 1.0)

            def emit_zero_acc(acc_tile):
                # zero a whole [128,512] PSUM bank with one K=1 matmul
                # (213ns on PE, arms+clears the full 2KB zero-region)
                nc.tensor.matmul(acc_tile[:], zeros_sb[0:1, 0:128],
                                 zeros_sb[0:1, :], start=True, stop=True,
                                 skip_group_check=True)

            def emit_scores_pair(sdst_a, sdst_b, pr, t, q0):
                nc.tensor.matmul(sdst_a,
                                 kT_sb[0:64, pr, t * 128:(t + 1) * 128],
                                 q_sb[0:64, pr, q0:q0 + QT],
                                 start=True, stop=True, tile_position=(0, 0))
                nc.tensor.matmul(sdst_b,
                                 kT_sb[64:128, pr, t * 128:(t + 1) * 128],
                                 q_sb[64:128, pr, q0:q0 + QT],
                                 start=True, stop=True, tile_position=(64, 0))

            def emit_attnv(accA, accB, ex, pr, t):
                # transposed attn@V: out[q,65] = ex[t, qblk].T @ vx[t, :].
                # The four mc accumulation groups share one PSUM bank, and a
                # start=True zeroes the WHOLE 2KB zero-region — so the acc
                # tile is memset once instead and every matmul accumulates
                # (start=False), which is also order-independent.
                hA, hB = 2 * pr, 2 * pr + 1
                for mc in range(4):
                    nc.tensor.matmul(accA[:, mc * 128:mc * 128 + D + 1],
                                     ex[:, mc * 128:(mc + 1) * 128],
                                     vx_sb[:, t, hA, :],
                                     start=False, stop=(t == TCH - 1),
                                     skip_group_check=True)
                for mc in range(4):
                    nc.tensor.matmul(accB[:, mc * 128:mc * 128 + D + 1],
                                     ex[:, QT + mc * 128:QT + (mc + 1) * 128],
                                     vx_sb[:, t, hB, :],
                                     start=False, stop=(t == TCH - 1),
                                     skip_group_check=True)

            def emit_dve_exp2(ex, s1, s2):
                """exp for a chunk whose scores live in two pwo bank tiles
                (keeps the main score ring off the DVE queue entirely)."""
                i16 = dvexp.tile([128, 2 * QT], I16, tag="i16", bufs=3, name="i16")
                nc.vector.tensor_scalar(i16[:, 0:QT], s1, EXP_SC, EXP_B0,
                                        ALU.mult, ALU.add)
                nc.vector.tensor_scalar(i16[:, QT:2 * QT], s2, EXP_SC, EXP_B0,
                                        ALU.mult, ALU.add)
                _dve_exp_tail(ex, i16)

            def _dve_exp_tail(ex, i16):
                e_lin = i16[:].bitcast(FP16)
                m16 = dvexp.tile([128, 2 * QT], I16, tag="m16", bufs=3, name="m16")
                nc.vector.tensor_scalar(m16[:], i16[:], 0x03FF, 0x3C00,
                                        ALU.bitwise_and, ALU.bitwise_or)
                m = m16[:].bitcast(FP16)
                t1 = dvexp.tile([128, 2 * QT], FP16, tag="t1", bufs=3, name="t1")
                nc.vector.tensor_scalar(t1[:], m, EXP_A2, EXP_A1,
                                        ALU.mult, ALU.add)
                t2 = dvexp.tile([128, 2 * QT], FP16, tag="t2", bufs=3, name="t2")
                nc.vector.tensor_tensor(t2[:], t1[:], m, ALU.mult)
                g = dvexp.tile([128, 2 * QT], FP16, tag="g", bufs=3, name="g")
                nc.vector.tensor_scalar(g[:], t2[:], EXP_A0, None, ALU.add)
                nc.vector.tensor_tensor(ex, e_lin, g[:], ALU.mult)

            def emit_dve_exp(ex, sc):
                """exp(sc*0.125) -> ex (bf16): the PSUM-touching op and the
                final mul on DVE, the middle of the chain on the idle Pool
                engine so DVE stays responsive (~1.8us/chunk instead of
                3.4us)."""
                i16 = dvexp.tile([128, 2 * QT], I16, tag="i16", bufs=3, name="i16")
                nc.vector.tensor_scalar(i16[:], sc, EXP_SC, EXP_B0,
                                        ALU.mult, ALU.add)
                e_lin = i16[:].bitcast(FP16)
                m16 = dvexp.tile([128, 2 * QT], I16, tag="m16", bufs=3, name="m16")
                nc.vector.tensor_scalar(m16[:], i16[:], 0x03FF, 0x3C00,
                                        ALU.bitwise_and, ALU.bitwise_or)
                m = m16[:].bitcast(FP16)
                t1 = dvexp.tile([128, 2 * QT], FP16, tag="t1", bufs=3, name="t1")
                nc.vector.tensor_scalar(t1[:], m, EXP_A2, EXP_A1,
                                        ALU.mult, ALU.add)
                t2 = dvexp.tile([128, 2 * QT], FP16, tag="t2", bufs=3, name="t2")
                nc.vector.tensor_tensor(t2[:], t1[:], m, ALU.mult)
                g = dvexp.tile([128, 2 * QT], FP16, tag="g", bufs=3, name="g")
                nc.vector.tensor_scalar(g[:], t2[:], EXP_A0, None, ALU.add)
                nc.vector.tensor_tensor(ex, e_lin, g[:], ALU.mult)

            def epilogue_unit(acc, nmT_by_mc, half, mc, tail=False):
                """normalize + transpose one (head, mc) block.
                acc: [128,512] psum (4 mc slices of [128,65]). The transpose
                output aliases the just-consumed acc slice (the nm-mul has
                already read it), so no extra PSUM is needed.
                half: 0/1 = which partition-half of nmT gets this head."""
                sl = acc[:, mc * 128:mc * 128 + D + 1]
                rc = rcp.tile([128, 1], F32, tag="rc", name="rc")
                nc.vector.reciprocal(rc[:], sl[:, D:D + 1])
                nm = nmp.tile([128, D], BF16, tag="nm", name="nm")
                nc.vector.tensor_scalar(nm[:], sl[:, 0:D], rc[:], None,
                                        ALU.mult)
                tslot = acc[0:64, mc * 128:mc * 128 + 64].bitcast(BF16)
                nc.tensor.transpose(tslot, nm[:], eye_sb[:])
                dst = nmT_by_mc[mc][64 * half:64 * half + 64, :]
                if tail:
                    nc.scalar.copy(dst, tslot)
                else:
                    nc.vector.tensor_copy(dst, tslot)

            def emit_epilogue(acc, nmT_by_mc, half):
                for mc in range(4):
                    epilogue_unit(acc, nmT_by_mc, half, mc)

            def new_nmT_set():
                return [nmtp.tile([128, 128], BF16, tag="nmt", name="nmt")
                        for _ in range(4)]

            # ============ PHASE 1: stream + projections + j0 chase ============
            # pool creation order fixes banks: chase accs (freed LAST, by j0
            # epilogues) on banks 0-3 where phase-2 accs go; kps/vps + chase
            # score ring (freed at stream end) on banks 4-7 where the phase-2
            # score ring goes.
            nmT_j0 = {0: None, 1: None}   # by pair
            with (
                tc.tile_pool(name="pacc", bufs=4, space="PSUM") as pacc,
                tc.tile_pool(name="pkv", bufs=1, space="PSUM") as pkv,
                tc.tile_pool(name="ps2", bufs=2, space="PSUM") as ps2,
            ):
                o_acc = {}
                for pr in range(2):
                    o_acc[pr] = (pacc.tile([128, QT], F32, tag="acc", name="oA"),
                                 pacc.tile([128, QT], F32, tag="acc", name="oB"))
                    emit_zero_acc(o_acc[pr][0])
                    emit_zero_acc(o_acc[pr][1])

                # critical-path DMA splitting (same as v1)
                kblk0 = stream.tile([128, 8, TB], BF16, tag="kblk", name="kblk0")
                nc.sync.dma_start(wk_sb[:, 0:1, :], wkT_r[:, 0:1, :])
                nc.sync.dma_start(kblk0[:, 0:1, :], keyT_r[:, 0:1, 0:TB])
                nc.sync.dma_start(wk_sb[:, 1:8, :], wkT_r[:, 1:8, :])
                nc.sync.dma_start(kblk0[:, 1:8, :], keyT_r[:, 1:8, 0:TB])
                nc.sync.dma_start(q_sb[:, :, 0:QT], qT_r[:, :, 0:QT])
                nc.sync.dma_start(wv_sb[:], wvT_r)

                def chase_scores(t, pr, q0=0):
                    s1 = ps2.tile([128, QT], F32, tag="ssc", name="s1")
                    s2 = ps2.tile([128, QT], F32, tag="ssc", name="s2")
                    emit_scores_pair(s1[:], s2[:], pr, t, q0)
                    ex = expp.tile([128, 2 * QT], BF16, tag="exp", name="ex")
                    nc.scalar.activation(ex[:, 0:QT], s1[:], EXPF, scale=0.125)
                    nc.scalar.activation(ex[:, QT:2 * QT], s2[:], EXPF, scale=0.125)
                    return ex

                ex_t = {}
                for tb in range(NTB):
                    ts0 = tb * TB
                    if tb == 0:
                        kblk = kblk0
                    else:
                        kblk = stream.tile([128, 8, TB], BF16, tag="kblk", name="kblk")
                        nc.sync.dma_start(kblk[:], keyT_r[:, :, ts0:ts0 + TB])
                    vblk = stream.tile([128, 8, TB], BF16, tag="vblk", name="vblk", bufs=4)
                    nc.sync.dma_start(vblk[:], valT_r[:, :, ts0:ts0 + TB])
                    kps = pkv.tile([128, 2, TB], F32, tag="kps", name="kps")
                    for mc in range(2):
                        for c in range(8):
                            nc.tensor.matmul(kps[:, mc, :], wk_sb[:, c, mc * 128:(mc + 1) * 128],
                                             kblk[:, c, :], start=(c == 0), stop=(c == 7))
                        nc.vector.tensor_copy(kT_sb[:, mc, ts0:ts0 + TB], kps[:, mc, :])
                        if tb > 0:
                            ex_t[(2 * tb - 1, mc)] = chase_scores(2 * tb - 1, mc)
                    vps = pkv.tile([128, 2, C], F32, tag="vps", name="vps")
                    for t2 in range(TB // 128):
                        for c in range(8):
                            nc.tensor.matmul(vps[:, t2, :], vblk[:, c, t2 * 128:(t2 + 1) * 128],
                                             wv_sb[:, c, :], start=(c == 0), stop=(c == 7))
                        tg = tb * (TB // 128) + t2
                        nc.vector.tensor_copy(
                            vx_sb[:, tg, :, 0:D],
                            vps[:, t2, :].rearrange("p (h d) -> p h d", h=HPC))
                        ex_t[(2 * tb, t2)] = chase_scores(2 * tb, t2)
                    for t in (2 * tb - 1, 2 * tb):
                        if t < 0:
                            continue
                        for pr in range(2):
                            emit_attnv(o_acc[pr][0], o_acc[pr][1],
                                       ex_t.pop((t, pr))[:], pr, t)
                # final odd chunk: score tiles borrow the freed projection banks
                tL = NK // 128 - 1
                for mc in range(2):
                    s1 = pkv.tile([128, QT], F32, tag="kps", name="s1t")
                    s2 = pkv.tile([128, QT], F32, tag="vps", name="s2t")
                    emit_scores_pair(s1[:], s2[:], mc, tL, 0)
                    exL = expp.tile([128, 2 * QT], BF16, tag="exp", name="exL")
                    nc.scalar.activation(exL[:, 0:QT], s1[:], EXPF, scale=0.125)
                    nc.scalar.activation(exL[:, QT:2 * QT], s2[:], EXPF, scale=0.125)
                    ex_t[(tL, mc)] = exL
                # phase-2 q tiles + W_o + identity load after the stream
                nc.sync.dma_start(q_sb[:, :, QT:NQ], qT_r[:, :, QT:NQ])
                nc.sync.dma_start(wo2_sb[:], wo2_d)
                nc.sync.dma_start(eye_sb[:], eye_d)
                for pr in range(2):
                    emit_attnv(o_acc[pr][0], o_acc[pr][1],
                               ex_t.pop((tL, pr))[:], pr, tL)
                # j0 epilogues (transposes alias the chase-acc slices).
                # Stage-major emission: all nm-muls back-to-back on DVE, then
                # all transposes on PE, then all copies on the idle ACT —
                # avoids per-unit cross-engine sem bubbles in DVE's in-order
                # stream.
                units = []
                for pr in range(2):
                    nmT_j0[pr] = new_nmT_set()
                    for half in range(2):
                        a_ = o_acc[pr][half]
                        for mc in range(4):
                            units.append((a_, nmT_j0[pr], half, mc))
                nms = []
                for (a_, nmT, half, mc) in units:
                    sl = a_[:, mc * 128:mc * 128 + D + 1]
                    rc = rcp.tile([128, 1], F32, tag="rc", name="rc")
                    nc.vector.reciprocal(rc[:], sl[:, D:D + 1])
                    nm = nmp.tile([128, D], BF16, tag="nm", name="nm")
                    nc.vector.tensor_scalar(nm[:], sl[:, 0:D], rc[:], None,
                                            ALU.mult)
                    nms.append(nm)
                for u, (a_, nmT, half, mc) in enumerate(units):
                    tslot = a_[0:64, mc * 128:mc * 128 + 64].bitcast(BF16)
                    nc.tensor.transpose(tslot, nms[u][:], eye_sb[:])
                for u, (a_, nmT, half, mc) in enumerate(units):
                    tslot = a_[0:64, mc * 128:mc * 128 + 64].bitcast(BF16)
                    nc.vector.tensor_copy(
                        nmT[mc][64 * half:64 * half + 64, :], tslot)

            # ================= PHASE 2: j1..j3 + all W_o =================
            with (
                tc.tile_pool(name="pacc2", bufs=2, space="PSUM") as pacc2,
                tc.tile_pool(name="pwo", bufs=2, space="PSUM") as pwo,
                tc.tile_pool(name="pscore", bufs=2, space="PSUM") as pscore,
            ):
                sweeps = [(j, pr) for j in range(1, NJ) for pr in range(2)]
                NS = len(sweeps)           # 6
                total = NS * TCH           # 192 chunk iterations
                FLUSH_T = 8                # early attnvs emitted at this t
                FLUSH_T0 = 14              # sweep 0 waits out the j0 epilogue

                acc = [None] * NS
                dve_chunks = set()
                deferred = {}
                early = [list() for _ in range(NS)]
                sc_t = [dict() for _ in range(NS)]
                ex_t = [dict() for _ in range(NS)]
                nmT_by_j = {0: nmT_j0, 1: {}, 2: {}, 3: {}}

                filler = []   # (kind, closure); kind: 'ep' cheap, 'wo' heavier

                def wo_micro_ops(j, tail=False):
                    if tail:
                        # tail: halves (N=512) instead of quarters — fewer
                        # cross-engine hops on the drain critical path
                        ops = []
                        for mc in range(4):
                            osb_holder = {}
                            for h2 in range(2):
                                def mkt(mc, h2, osb_holder):
                                    def fn():
                                        psw = pwo.tile([128, QT], F32,
                                                       tag="wps", name="pswt")
                                        for p in range(2):
                                            nc.tensor.matmul(
                                                psw[:],
                                                nmT_by_j[j][p][mc][:],
                                                wo2_sb[:, p,
                                                       h2 * QT:(h2 + 1) * QT],
                                                start=(p == 0), stop=(p == 1))
                                        if h2 == 0:
                                            osb_holder[0] = outsb.tile(
                                                [128, E], F32, tag="osb",
                                                name="osb")
                                        osb = osb_holder[0]
                                        nc.scalar.copy(
                                            osb[:, h2 * QT:(h2 + 1) * QT],
                                            psw[:])
                                        nc.sync.dma_start(
                                            out_d[j * QT + mc * 128:
                                                  j * QT + (mc + 1) * 128,
                                                  h2 * QT:(h2 + 1) * QT],
                                            osb[:, h2 * QT:(h2 + 1) * QT])
                                    return fn
                                ops.append(("wo", mkt(mc, h2, osb_holder)))
                        return ops
                    # one unit per (mc, e-quarter): 2 pair-matmuls (N=256,
                    # 107ns each) into a half-bank psw (ring of 2 so the next
                    # unit's matmuls overlap this unit's copy) + osb copy;
                    # DMA the row block after its 4th quarter lands.
                    ops = []
                    for mc in range(4):
                        osb_holder = {}
                        for q4 in range(4):
                            def mk(mc, q4, osb_holder, tail=tail):
                                def fn():
                                    psw = pwo.tile([128, 256], F32, tag="wps",
                                                   name="psw")
                                    for p in range(2):
                                        nc.tensor.matmul(
                                            psw[:],
                                            nmT_by_j[j][p][mc][:],
                                            wo2_sb[:, p, q4 * 256:(q4 + 1) * 256],
                                            start=(p == 0), stop=(p == 1))
                                    if q4 == 0:
                                        osb_holder[0] = outsb.tile(
                                            [128, E], F32, tag="osb", name="osb")
                                    osb = osb_holder[0]
                                    if tail:
                                        nc.scalar.copy(
                                            osb[:, q4 * 256:(q4 + 1) * 256],
                                            psw[:])
                                    else:
                                        nc.vector.tensor_copy(
                                            osb[:, q4 * 256:(q4 + 1) * 256],
                                            psw[:])
                                    if q4 == 3:
                                        nc.sync.dma_start(
                                            out_d[j * QT + mc * 128:
                                                  j * QT + (mc + 1) * 128, :],
                                            osb[:])
                                return fn
                            ops.append(("wo", mk(mc, q4, osb_holder)))
                    return ops

                def sweep_of(i):
                    return i // TCH, i % TCH

                for i in range(total + 2):
                    # ---- stage A: scores(i) ----
                    if i < total:
                        s, t = sweep_of(i)
                        j, pr = sweeps[s]
                        if t == 0:
                            oA = pacc2.tile([128, QT], F32, tag="acc", name="oA")
                            oB = pacc2.tile([128, QT], F32, tag="acc", name="oB")
                            acc[s] = (oA, oB)
                        if dve_exp_every and ((pr == 1 and t in (8, 11, 14, 17, 20, 23, 26)) or (pr == 0 and t in (23, 26))):
                            sd1 = pwo.tile([128, QT], F32, tag="wps", name="sd1")
                            sd2 = pwo.tile([128, QT], F32, tag="wps", name="sd2")
                            emit_scores_pair(sd1[:], sd2[:], pr, t, j * QT)
                            sc_t[s][t] = (sd1, sd2)
                        else:
                            sc = pscore.tile([128, 2 * QT], F32, tag="sc",
                                             name="sc")
                            emit_scores_pair(sc[:, 0:QT], sc[:, QT:2 * QT], pr,
                                             t, j * QT)
                            sc_t[s][t] = sc
                    # ---- stage B: exp(i-1) ----
                    if 1 <= i <= total:
                        s, t = sweep_of(i - 1)
                        sc = sc_t[s].pop(t)
                        ex = expp.tile([128, 2 * QT], BF16, tag="exp", name="ex")
                        if isinstance(sc, tuple):
                            emit_dve_exp2(ex[:], sc[0][:], sc[1][:])
                            dve_chunks.add((s, t))
                        else:
                            nc.scalar.activation(ex[:], sc[:], EXPF, scale=0.125)
                        ex_t[s][t] = ex
                    # ---- deferred attnv of DVE-exp chunks (the PE stream is
                    # in-order: emitting these at i-2 would stall PE ~2.3us
                    # behind the 3.4us DVE chain; accumulation order is free
                    # thanks to zeroing+start=False, so emit 4 iters late) ----
                    if i in deferred:
                        for (s2, t2, ex2) in deferred.pop(i):
                            j2, pr2 = sweeps[s2]
                            emit_attnv(acc[s2][0], acc[s2][1], ex2[:], pr2, t2)
                    # ---- stage C: attnv(i-2) ----
                    if i >= 2:
                        s, t = sweep_of(i - 2)
                        j, pr = sweeps[s]
                        ex = ex_t[s].pop(t)
                        ft = FLUSH_T0 if s == 0 else FLUSH_T
                        if t < ft:
                            # held until the previous sweep's paced epilogue
                            # has consumed the acc banks this sweep reuses
                            early[s].append((t, ex))
                        elif t == ft:
                            emit_zero_acc(acc[s][0])
                            emit_zero_acc(acc[s][1])
                            for (t2, ex2) in early[s]:
                                emit_attnv(acc[s][0], acc[s][1], ex2[:], pr, t2)
                            early[s] = []
                            emit_attnv(acc[s][0], acc[s][1], ex[:], pr, t)
                        elif (s, t) in dve_chunks:
                            deferred.setdefault(i + 10, []).append((s, t, ex))
                        else:
                            emit_attnv(acc[s][0], acc[s][1], ex[:], pr, t)
                        if t == TCH - 1:
                            # flush any still-deferred attnvs of this sweep
                            # before its epilogue reads the accumulators
                            for rel in sorted(deferred):
                                keep = []
                                for (s2, t2, ex2) in deferred[rel]:
                                    if s2 == s:
                                        emit_attnv(acc[s][0], acc[s][1],
                                                   ex2[:], pr, t2)
                                    else:
                                        keep.append((s2, t2, ex2))
                                if keep:
                                    deferred[rel] = keep
                                else:
                                    del deferred[rel]
                            if pr == 0:
                                nmT_by_j[j] = {0: None, 1: None}
                            nmT_by_j[j][pr] = new_nmT_set()
                            accA, accB = acc[s]
                            nmT = nmT_by_j[j][pr]
                            for mc_ in range(4):
                                for half_, a_ in ((0, accA), (1, accB)):
                                    tail_ = False
                                    def mk_ep(a_, half_, mc_, nmT=nmT,
                                              tail_=tail_):
                                        def fn():
                                            epilogue_unit(a_, nmT, half_, mc_,
                                                          tail=tail_)
                                        return fn
                                    filler.append(("ep", mk_ep(a_, half_, mc_)))
                            acc[s] = None
                            if pr == 1:
                                filler.extend(wo_micro_ops(j, tail=(s == NS - 1)))
                    if i == 4:
                        filler.extend(wo_micro_ops(0))
                    # ---- stage D: paced epilogue/W_o micro-ops: epilogues
                    # 2/iteration; W_o units only on even iterations so their
                    # psw/copy chain never bunches against the DVE queue ----
                    if filler:
                        if filler[0][0] == "ep":
                            filler.pop(0)[1]()
                            if filler and filler[0][0] == "ep":
                                filler.pop(0)[1]()
                        elif i % 2 == 0:
                            filler.pop(0)[1]()

                for kind, fn in filler:
                    fn()

    nc.compile()
    return nc


_nc = None


def kernel(query, key, value, W_k, W_v, W_o):
    global _nc, _last_results, _last_in_maps
    if _nc is None:
        _nc = _build()

    import ml_dtypes
    query = np.asarray(query, dtype=np.float32)
    key = np.asarray(key, dtype=np.float32)
    value = np.asarray(value, dtype=np.float32)
    W_k = np.asarray(W_k, dtype=np.float32)
    W_v = np.asarray(W_v, dtype=np.float32)
    W_o = np.asarray(W_o, dtype=np.float32)

    keyT = [np.ascontiguousarray(key[b].T).astype(ml_dtypes.bfloat16)
            for b in range(B)]
    valT = [np.ascontiguousarray(value[b].T).astype(ml_dtypes.bfloat16)
            for b in range(B)]
    eye = np.eye(128, dtype=np.float32).astype(ml_dtypes.bfloat16)

    in_maps = []
    for b in range(B):
        for g in range(4):
            c0 = g * C
            wo2 = np.ascontiguousarray(
                W_o[:, c0:c0 + C].T.reshape(2, 128, E).transpose(1, 0, 2)
            ).astype(ml_dtypes.bfloat16)
            in_maps.append({
                "keyT": keyT[b],
                "valT": valT[b],
                "qT": np.ascontiguousarray(query[b][:, c0:c0 + C].T),
                "wkT": np.ascontiguousarray(
                    W_k[c0:c0 + C, :].T).astype(ml_dtypes.bfloat16),
                "wvT": np.ascontiguousarray(
                    W_v[c0:c0 + C, :].T).astype(ml_dtypes.bfloat16),
                "wo2": wo2,
                "eye": eye,
            })

    _last_in_maps = in_maps
    res = run_bass_kernel_spmd(
        _nc, in_maps, core_ids=list(range(8)),
        trace=bool(os.environ.get("BASS_TRACE")))
    _last_results = res

    out = np.zeros((B, NQ, E), dtype=np.float32)
    for b in range(B):
        for g in range(4):
            out[b] += res.results[b * 4 + g]["out"]
    return out


# revision 59
# speedup vs baseline: 1.0041x; 1.0007x over previous
"""MultiHeadAttention Trainium2 kernel (8 NeuronCores), v2.

Reference computation (torch-style Linear, x @ W.T):
    k = key @ W_k.T; v = value @ W_v.T; q = query (no projection)
    scores = q @ k.T / sqrt(64) per head; attn = softmax(scores)
    out = (attn @ v) @ W_o.T

Sharding: core = (batch b, head-group g); each core owns 4 heads of one
batch. Projection weights are column-split by head; the final W_o matmul
is a partial sum over the core's 256 head-channels, summed on host.

v2 keys off the cost model's matmul law (time = out_free_size x
cycles_per_row, independent of K/M):
  - attn@V is TRANSPOSED: out[q,65] = exp[t,q-block].T @ v_ext[t,65],
    putting 128 q's (not 65 dims) on PSUM partitions -> 2.05x fewer PE
    cycles than the [65,q] orientation.
  - softmax denominator: column 64 of the transposed accumulator; the
    epilogue is recip + per-partition broadcast mul (DVE, free-dim ops,
    no gpsimd partition_broadcast needed).
  - normalized heads are PE-transposed back ([128q,64] -> [64,128q],
    128 cycles each) and stacked in head-PAIRS so W_o runs with K=128:
    half the W_o matmuls of the K=64 version; bf16 weights.
  - K/V (and W_k/W_v) stream from DRAM as bf16: phase 1 was DMA-bound
    at ~99us for the f32 stream; halving the bytes makes it PE-bound
    (~96us of projections + j0 chase) at 95-97% occupancy.
  - 27 of 192 phase-2 exp chunks run on DVE instead of ScalarE via an
    fp16-bit-space Schraudolph + quadratic mantissa correction
    (max rel err 0.9%, rms 0.26% vs 0.39%/0.17% for the ACT path).
    Their scores route through the W_o PSUM banks and their attnvs are
    emitted 10 iterations late, so neither the score ring nor the
    in-order PE stream ever waits on the 3.4us DVE chain. Offload slots
    sit in W_o-quiet windows (pr1 sweeps t in {8,11,14,17,20,23,26},
    pr0 t in {23,26}); denser placement congests DVE and loses time.

Schedule: phase 1 streams K/V + projections with two chase sweeps (j0,
both head pairs) like v1; j0 epilogues are stage-batched to avoid
cross-engine bubbles in DVE's in-order stream. Phase 2 runs 6 sweeps
software-pipelined (scores(i) -> exp(i-1) -> attnv(i-2)) with ScalarE
measured at 98-100% busy; each sweep's first 8 attnvs are held until
t=8 (acc banks ring through 2 banks; a K=1 zero-matmul re-arms each
bank in 213ns on PE); epilogue and W_o micro-ops are paced 1-2 per
iteration through dedicated half-bank W_o PSUM slots.
"""

import os
import numpy as np

import concourse.bacc as bacc
import concourse.tile as tile
import concourse.mybir as mybir
from concourse.bass_utils import run_bass_kernel_spmd

F32 = mybir.dt.float32
F32R = mybir.dt.float32r
BF16 = mybir.dt.bfloat16
FP16 = mybir.dt.float16
I16 = mybir.dt.int16
EXPF = mybir.ActivationFunctionType.Exp
ALU = mybir.AluOpType

B, NQ, NK, E, H, D = 2, 2048, 4096, 1024, 16, 64
HPC = 4          # heads per core
C = HPC * D      # head-channels per core (256)
TB = 256         # token block for streaming K/V projections
NTB = NK // TB   # 16
TCH = NK // 128  # 32 t-chunks for attention
QT = 512         # q tile
NJ = NQ // QT    # 4

# ---- DVE softmax-exp (fp16-bit-space Schraudolph + quad correction) ----
# exp(s/8) = 2^y, y = s * (log2e/8). i16 = round(y*1024 + 15*1024) gives
# the fp16 bit pattern of 2^k*(1+f) (k=int(y), f=frac(y)); the quadratic
# g(m) ~= 2^(m-1)/m on m in [1,2) corrects the linear mantissa.
EXP_SC = float(0.125 * np.log2(np.e) * 1024.0)
EXP_B0 = float(15.0 * 1024.0)
_mm = (np.arange(1024) / 1024.0 + 1.0).astype(np.float64)
_G = 2.0 ** (_mm - 1) / _mm
_ch = np.polynomial.chebyshev.Chebyshev.fit(_mm, _G, 2)
_p = _ch.convert(kind=np.polynomial.Polynomial)
EXP_A0, EXP_A1, EXP_A2 = [float(v) for v in _p.coef]

_last_results = None
_last_in_maps = None


def _build(dve_exp_every=1):
    """dve_exp_every: in phase 2, every Nth chunk's exp runs on DVE
    (0 = never)."""
    nc = bacc.Bacc("TRN2", target_bir_lowering=False, debug=False, num_devices=8)

    keyT_d = nc.dram_tensor("keyT", [E, NK], BF16, kind="ExternalInput").ap()
    valT_d = nc.dram_tensor("valT", [E, NK], BF16, kind="ExternalInput").ap()
    qT_d = nc.dram_tensor("qT", [C, NQ], F32, kind="ExternalInput").ap()
    wkT_d = nc.dram_tensor("wkT", [E, C], BF16, kind="ExternalInput").ap()
    wvT_d = nc.dram_tensor("wvT", [E, C], BF16, kind="ExternalInput").ap()
    wo2_d = nc.dram_tensor("wo2", [128, 2, E], BF16, kind="ExternalInput").ap()
    eye_d = nc.dram_tensor("eye", [128, 128], BF16, kind="ExternalInput").ap()
    out_d = nc.dram_tensor("out", [NQ, E], F32, kind="ExternalOutput").ap()

    keyT_r = keyT_d.rearrange("(c p) n -> p c n", p=128)
    valT_r = valT_d.rearrange("(c p) n -> p c n", p=128)
    qT_r = qT_d.rearrange("(c p) n -> p c n", p=128).bitcast(F32R)
    wkT_r = wkT_d.rearrange("(c p) n -> p c n", p=128)
    wvT_r = wvT_d.rearrange("(c p) n -> p c n", p=128)

    with tile.TileContext(nc) as tc:
        with (
            tc.tile_pool(name="wpool", bufs=1) as wpool,
            tc.tile_pool(name="stream", bufs=3) as stream,
            tc.tile_pool(name="big", bufs=1) as big,
            tc.tile_pool(name="expp", bufs=14) as expp,
            tc.tile_pool(name="nmp", bufs=12) as nmp,
            tc.tile_pool(name="nmtp", bufs=12) as nmtp,
            tc.tile_pool(name="rcp", bufs=6) as rcp,
            tc.tile_pool(name="outsb", bufs=3) as outsb,
            tc.tile_pool(name="dvexp", bufs=4) as dvexp,
        ):
            # ---- resident weights / q ----
            wk_sb = wpool.tile([128, 8, C], BF16)
            wv_sb = wpool.tile([128, 8, C], BF16)
            wo2_sb = wpool.tile([128, 2, E], BF16)
            eye_sb = wpool.tile([128, 128], BF16)
            q_sb = wpool.tile([128, 2, NQ], F32R)

            # ---- resident kT / v_ext ----
            kT_sb = big.tile([128, 2, NK], F32R)            # [hd%128, hd//128, t]
            vx_sb = big.tile([128, TCH, HPC, D + 1], BF16)  # [t%128, t//128, h, d|1]
            zeros_sb = big.tile([1, QT], BF16)
            nc.vector.memset(zeros_sb[:], 0.0)
            for t in range(TCH):
                nc.gpsimd.memset(vx_sb[:, t, :, D:D + 1], 1.0)

            def emit_zero_acc(acc_tile):
                # zero a whole [128,512] PSUM bank with one K=1 matmul
                # (213ns on PE, arms+clears the full 2KB zero-region)
                nc.tensor.matmul(acc_tile[:], zeros_sb[0:1, 0:128],
                                 zeros_sb[0:1, :], start=True, stop=True,
                                 skip_group_check=True)

            def emit_scores_pair(sdst_a, sdst_b, pr, t, q0):
                nc.tensor.matmul(sdst_a,
                                 kT_sb[0:64, pr, t * 128:(t + 1) * 128],
                                 q_sb[0:64, pr, q0:q0 + QT],
                                 start=True, stop=True, tile_position=(0, 0))
                nc.tensor.matmul(sdst_b,
                                 kT_sb[64:128, pr, t * 128:(t + 1) * 128],
                                 q_sb[64:128, pr, q0:q0 + QT],
                                 start=True, stop=True, tile_position=(64, 0))

            def emit_attnv(accA, accB, ex, pr, t):
                # transposed attn@V: out[q,65] = ex[t, qblk].T @ vx[t, :].
                # The four mc accumulation groups share one PSUM bank, and a
                # start=True zeroes the WHOLE 2KB zero-region — so the acc
                # tile is memset once instead and every matmul accumulates
                # (start=False), which is also order-independent.
                hA, hB = 2 * pr, 2 * pr + 1
                for mc in range(4):
                    nc.tensor.matmul(accA[:, mc * 128:mc * 128 + D + 1],
                                     ex[:, mc * 128:(mc + 1) * 128],
                                     vx_sb[:, t, hA, :],
                                     start=False, stop=(t == TCH - 1),
                                     skip_group_check=True)
                for mc in range(4):
                    nc.tensor.matmul(accB[:, mc * 128:mc * 128 + D + 1],
                                     ex[:, QT + mc * 128:QT + (mc + 1) * 128],
                                     vx_sb[:, t, hB, :],
                                     start=False, stop=(t == TCH - 1),
                                     skip_group_check=True)

            def emit_dve_exp2(ex, s1, s2):
                """exp for a chunk whose scores live in two pwo bank tiles
                (keeps the main score ring off the DVE queue entirely)."""
                i16 = dvexp.tile([128, 2 * QT], I16, tag="i16", bufs=3, name="i16")
                nc.vector.tensor_scalar(i16[:, 0:QT], s1, EXP_SC, EXP_B0,
                                        ALU.mult, ALU.add)
                nc.vector.tensor_scalar(i16[:, QT:2 * QT], s2, EXP_SC, EXP_B0,
                                        ALU.mult, ALU.add)
                _dve_exp_tail(ex, i16)

            def _dve_exp_tail(ex, i16):
                e_lin = i16[:].bitcast(FP16)
                m16 = dvexp.tile([128, 2 * QT], I16, tag="m16", bufs=3, name="m16")
                nc.vector.tensor_scalar(m16[:], i16[:], 0x03FF, 0x3C00,
                                        ALU.bitwise_and, ALU.bitwise_or)
                m = m16[:].bitcast(FP16)
                t1 = dvexp.tile([128, 2 * QT], FP16, tag="t1", bufs=3, name="t1")
                nc.vector.tensor_scalar(t1[:], m, EXP_A2, EXP_A1,
                                        ALU.mult, ALU.add)
                t2 = dvexp.tile([128, 2 * QT], FP16, tag="t2", bufs=3, name="t2")
                nc.vector.tensor_tensor(t2[:], t1[:], m, ALU.mult)
                g = dvexp.tile([128, 2 * QT], FP16, tag="g", bufs=3, name="g")
                nc.vector.tensor_scalar(g[:], t2[:], EXP_A0, None, ALU.add)
                nc.vector.tensor_tensor(ex, e_lin, g[:], ALU.mult)

            def emit_dve_exp(ex, sc):
                """exp(sc*0.125) -> ex (bf16): the PSUM-touching op and the
                final mul on DVE, the middle of the chain on the idle Pool
                engine so DVE stays responsive (~1.8us/chunk instead of
                3.4us)."""
                i16 = dvexp.tile([128, 2 * QT], I16, tag="i16", bufs=3, name="i16")
                nc.vector.tensor_scalar(i16[:], sc, EXP_SC, EXP_B0,
                                        ALU.mult, ALU.add)
                e_lin = i16[:].bitcast(FP16)
                m16 = dvexp.tile([128, 2 * QT], I16, tag="m16", bufs=3, name="m16")
                nc.vector.tensor_scalar(m16[:], i16[:], 0x03FF, 0x3C00,
                                        ALU.bitwise_and, ALU.bitwise_or)
                m = m16[:].bitcast(FP16)
                t1 = dvexp.tile([128, 2 * QT], FP16, tag="t1", bufs=3, name="t1")
                nc.vector.tensor_scalar(t1[:], m, EXP_A2, EXP_A1,
                                        ALU.mult, ALU.add)
                t2 = dvexp.tile([128, 2 * QT], FP16, tag="t2", bufs=3, name="t2")
                nc.vector.tensor_tensor(t2[:], t1[:], m, ALU.mult)
                g = dvexp.tile([128, 2 * QT], FP16, tag="g", bufs=3, name="g")
                nc.vector.tensor_scalar(g[:], t2[:], EXP_A0, None, ALU.add)
                nc.vector.tensor_tensor(ex, e_lin, g[:], ALU.mult)

            def epilogue_unit(acc, nmT_by_mc, half, mc, tail=False):
                """normalize + transpose one (head, mc) block.
                acc: [128,512] psum (4 mc slices of [128,65]). The transpose
                output aliases the just-consumed acc slice (the nm-mul has
                already read it), so no extra PSUM is needed.
                half: 0/1 = which partition-half of nmT gets this head."""
                sl = acc[:, mc * 128:mc * 128 + D + 1]
                rc = rcp.tile([128, 1], F32, tag="rc", name="rc")
                nc.vector.reciprocal(rc[:], sl[:, D:D + 1])
                nm = nmp.tile([128, D], BF16, tag="nm", name="nm")
                nc.vector.tensor_scalar(nm[:], sl[:, 0:D], rc[:], None,
                                        ALU.mult)
                tslot = acc[0:64, mc * 128:mc * 128 + 64].bitcast(BF16)
                nc.tensor.transpose(tslot, nm[:], eye_sb[:])
                dst = nmT_by_mc[mc][64 * half:64 * half + 64, :]
                if tail:
                    nc.scalar.copy(dst, tslot)
                else:
                    nc.vector.tensor_copy(dst, tslot)

            def emit_epilogue(acc, nmT_by_mc, half):
                for mc in range(4):
                    epilogue_unit(acc, nmT_by_mc, half, mc)

            def new_nmT_set():
                return [nmtp.tile([128, 128], BF16, tag="nmt", name="nmt")
                        for _ in range(4)]

            # ============ PHASE 1: stream + projections + j0 chase ============
            # pool creation order fixes banks: chase accs (freed LAST, by j0
            # epilogues) on banks 0-3 where phase-2 accs go; kps/vps + chase
            # score ring (freed at stream end) on banks 4-7 where the phase-2
            # score ring goes.
            nmT_j0 = {0: None, 1: None}   # by pair
            with (
                tc.tile_pool(name="pacc", bufs=4, space="PSUM") as pacc,
                tc.tile_pool(name="pkv", bufs=1, space="PSUM") as pkv,
                tc.tile_pool(name="ps2", bufs=2, space="PSUM") as ps2,
            ):
                o_acc = {}
                for pr in range(2):
                    o_acc[pr] = (pacc.tile([128, QT], F32, tag="acc", name="oA"),
                                 pacc.tile([128, QT], F32, tag="acc", name="oB"))
                    emit_zero_acc(o_acc[pr][0])
                    emit_zero_acc(o_acc[pr][1])

                # critical-path DMA splitting (same as v1)
                kblk0 = stream.tile([128, 8, TB], BF16, tag="kblk", name="kblk0")
                nc.sync.dma_start(wk_sb[:, 0:1, :], wkT_r[:, 0:1, :])
                nc.sync.dma_start(kblk0[:, 0:1, :], keyT_r[:, 0:1, 0:TB])
                nc.sync.dma_start(wk_sb[:, 1:8, :], wkT_r[:, 1:8, :])
                nc.sync.dma_start(kblk0[:, 1:8, :], keyT_r[:, 1:8, 0:TB])
                nc.sync.dma_start(q_sb[:, :, 0:QT], qT_r[:, :, 0:QT])
                nc.sync.dma_start(wv_sb[:], wvT_r)

                def chase_scores(t, pr, q0=0):
                    s1 = ps2.tile([128, QT], F32, tag="ssc", name="s1")
                    s2 = ps2.tile([128, QT], F32, tag="ssc", name="s2")
                    emit_scores_pair(s1[:], s2[:], pr, t, q0)
                    ex = expp.tile([128, 2 * QT], BF16, tag="exp", name="ex")
                    nc.scalar.activation(ex[:, 0:QT], s1[:], EXPF, scale=0.125)
                    nc.scalar.activation(ex[:, QT:2 * QT], s2[:], EXPF, scale=0.125)
                    return ex

                ex_t = {}
                for tb in range(NTB):
                    ts0 = tb * TB
                    if tb == 0:
                        kblk = kblk0
                    else:
                        kblk = stream.tile([128, 8, TB], BF16, tag="kblk", name="kblk")
                        nc.sync.dma_start(kblk[:], keyT_r[:, :, ts0:ts0 + TB])
                    vblk = stream.tile([128, 8, TB], BF16, tag="vblk", name="vblk", bufs=4)
                    nc.sync.dma_start(vblk[:], valT_r[:, :, ts0:ts0 + TB])
                    kps = pkv.tile([128, 2, TB], F32, tag="kps", name="kps")
                    for mc in range(2):
                        for c in range(8):
                            nc.tensor.matmul(kps[:, mc, :], wk_sb[:, c, mc * 128:(mc + 1) * 128],
                                             kblk[:, c, :], start=(c == 0), stop=(c == 7))
                        nc.vector.tensor_copy(kT_sb[:, mc, ts0:ts0 + TB], kps[:, mc, :])
                        if tb > 0:
                            ex_t[(2 * tb - 1, mc)] = chase_scores(2 * tb - 1, mc)
                    vps = pkv.tile([128, 2, C], F32, tag="vps", name="vps")
                    for t2 in range(TB // 128):
                        for c in range(8):
                            nc.tensor.matmul(vps[:, t2, :], vblk[:, c, t2 * 128:(t2 + 1) * 128],
                                             wv_sb[:, c, :], start=(c == 0), stop=(c == 7))
                        tg = tb * (TB // 128) + t2
                        nc.vector.tensor_copy(
                            vx_sb[:, tg, :, 0:D],
                            vps[:, t2, :].rearrange("p (h d) -> p h d", h=HPC))
                        ex_t[(2 * tb, t2)] = chase_scores(2 * tb, t2)
                    for t in (2 * tb - 1, 2 * tb):
                        if t < 0:
                            continue
                        for pr in range(2):
                            emit_attnv(o_acc[pr][0], o_acc[pr][1],
                                       ex_t.pop((t, pr))[:], pr, t)
                # final odd chunk: score tiles borrow the freed projection banks
                tL = NK // 128 - 1
                for mc in range(2):
                    s1 = pkv.tile([128, QT], F32, tag="kps", name="s1t")
                    s2 = pkv.tile([128, QT], F32, tag="vps", name="s2t")
                    emit_scores_pair(s1[:], s2[:], mc, tL, 0)
                    exL = expp.tile([128, 2 * QT], BF16, tag="exp", name="exL")
                    nc.scalar.activation(exL[:, 0:QT], s1[:], EXPF, scale=0.125)
                    nc.scalar.activation(exL[:, QT:2 * QT], s2[:], EXPF, scale=0.125)
                    ex_t[(tL, mc)] = exL
                # phase-2 q tiles + W_o + identity load after the stream
                nc.sync.dma_start(q_sb[:, :, QT:NQ], qT_r[:, :, QT:NQ])
                nc.sync.dma_start(wo2_sb[:], wo2_d)
                nc.sync.dma_start(eye_sb[:], eye_d)
                for pr in range(2):
                    emit_attnv(o_acc[pr][0], o_acc[pr][1],
                               ex_t.pop((tL, pr))[:], pr, tL)
                # j0 epilogues (transposes alias the chase-acc slices).
                # Stage-major emission: all nm-muls back-to-back on DVE, then
                # all transposes on PE, then all copies on the idle ACT —
                # avoids per-unit cross-engine sem bubbles in DVE's in-order
                # stream.
                units = []
                for pr in range(2):
                    nmT_j0[pr] = new_nmT_set()
                    for half in range(2):
                        a_ = o_acc[pr][half]
                        for mc in range(4):
                            units.append((a_, nmT_j0[pr], half, mc))
                nms = []
                for (a_, nmT, half, mc) in units:
                    sl = a_[:, mc * 128:mc * 128 + D + 1]
                    rc = rcp.tile([128, 1], F32, tag="rc", name="rc")
                    nc.vector.reciprocal(rc[:], sl[:, D:D + 1])
                    nm = nmp.tile([128, D], BF16, tag="nm", name="nm")
                    nc.vector.tensor_scalar(nm[:], sl[:, 0:D], rc[:], None,
                                            ALU.mult)
                    nms.append(nm)
                for u, (a_, nmT, half, mc) in enumerate(units):
                    tslot = a_[0:64, mc * 128:mc * 128 + 64].bitcast(BF16)
                    nc.tensor.transpose(tslot, nms[u][:], eye_sb[:])
                for u, (a_, nmT, half, mc) in enumerate(units):
                    tslot = a_[0:64, mc * 128:mc * 128 + 64].bitcast(BF16)
                    nc.vector.tensor_copy(
                        nmT[mc][64 * half:64 * half + 64, :], tslot)

            # ================= PHASE 2: j1..j3 + all W_o =================
            with (
                tc.tile_pool(name="pacc2", bufs=2, space="PSUM") as pacc2,
                tc.tile_pool(name="pwo", bufs=2, space="PSUM") as pwo,
                tc.tile_pool(name="pscore", bufs=2, space="PSUM") as pscore,
            ):
                sweeps = [(j, pr) for j in range(1, NJ) for pr in range(2)]
                NS = len(sweeps)           # 6
                total = NS * TCH           # 192 chunk iterations
                FLUSH_T = 8                # early attnvs emitted at this t
                FLUSH_T0 = 14              # sweep 0 waits out the j0 epilogue

                acc = [None] * NS
                dve_chunks = set()
                deferred = {}
                early = [list() for _ in range(NS)]
                sc_t = [dict() for _ in range(NS)]
                ex_t = [dict() for _ in range(NS)]
                nmT_by_j = {0: nmT_j0, 1: {}, 2: {}, 3: {}}

                filler = []   # (kind, closure); kind: 'ep' cheap, 'wo' heavier

                def wo_micro_ops(j, tail=False):
                    if tail:
                        # tail: halves (N=512) instead of quarters — fewer
                        # cross-engine hops on the drain critical path
                        ops = []
                        for mc in range(4):
                            osb_holder = {}
                            for h2 in range(2):
                                def mkt(mc, h2, osb_holder):
                                    def fn():
                                        psw = pwo.tile([128, QT], F32,
                                                       tag="wps", name="pswt")
                                        for p in range(2):
                                            nc.tensor.matmul(
                                                psw[:],
                                                nmT_by_j[j][p][mc][:],
                                                wo2_sb[:, p,
                                                       h2 * QT:(h2 + 1) * QT],
                                                start=(p == 0), stop=(p == 1))
                                        if h2 == 0:
                                            osb_holder[0] = outsb.tile(
                                                [128, E], F32, tag="osb",
                                                name="osb")
                                        osb = osb_holder[0]
                                        nc.scalar.copy(
                                            osb[:, h2 * QT:(h2 + 1) * QT],
                                            psw[:])
                                        nc.sync.dma_start(
                                            out_d[j * QT + mc * 128:
                                                  j * QT + (mc + 1) * 128,
                                                  h2 * QT:(h2 + 1) * QT],
                                            osb[:, h2 * QT:(h2 + 1) * QT])
                                    return fn
                                ops.append(("wo", mkt(mc, h2, osb_holder)))
                        return ops
                    # one unit per (mc, e-quarter): 2 pair-matmuls (N=256,
                    # 107ns each) into a half-bank psw (ring of 2 so the next
                    # unit's matmuls overlap this unit's copy) + osb copy;
                    # DMA the row block after its 4th quarter lands.
                    ops = []
                    for mc in range(4):
                        osb_holder = {}
                        for q4 in range(4):
                            def mk(mc, q4, osb_holder, tail=tail):
                                def fn():
                                    psw = pwo.tile([128, 256], F32, tag="wps",
                                                   name="psw")
                                    for p in range(2):
                                        nc.tensor.matmul(
                                            psw[:],
                                            nmT_by_j[j][p][mc][:],
                                            wo2_sb[:, p, q4 * 256:(q4 + 1) * 256],
                                            start=(p == 0), stop=(p == 1))
                                    if q4 == 0:
                                        osb_holder[0] = outsb.tile(
                                            [128, E], F32, tag="osb", name="osb")
                                    osb = osb_holder[0]
                                    if tail:
                                        nc.scalar.copy(
                                            osb[:, q4 * 256:(q4 + 1) * 256],
                                            psw[:])
                                    else:
                                        nc.vector.tensor_copy(
                                            osb[:, q4 * 256:(q4 + 1) * 256],
                                            psw[:])
                                    nc.sync.dma_start(
                                        out_d[j * QT + mc * 128:
                                              j * QT + (mc + 1) * 128,
                                              q4 * 256:(q4 + 1) * 256],
                                        osb[:, q4 * 256:(q4 + 1) * 256])
                                return fn
                            ops.append(("wo", mk(mc, q4, osb_holder)))
                    return ops

                def sweep_of(i):
                    return i // TCH, i % TCH

                for i in range(total + 2):
                    # ---- stage A: scores(i) ----
                    if i < total:
                        s, t = sweep_of(i)
                        j, pr = sweeps[s]
                        if t == 0:
                            oA = pacc2.tile([128, QT], F32, tag="acc", name="oA")
                            oB = pacc2.tile([128, QT], F32, tag="acc", name="oB")
                            acc[s] = (oA, oB)
                        if dve_exp_every and ((pr == 1 and t in (8, 11, 14, 17, 20, 23, 26)) or (pr == 0 and t in (23, 26))):
                            sd1 = pwo.tile([128, QT], F32, tag="wps", name="sd1")
                            sd2 = pwo.tile([128, QT], F32, tag="wps", name="sd2")
                            emit_scores_pair(sd1[:], sd2[:], pr, t, j * QT)
                            sc_t[s][t] = (sd1, sd2)
                        else:
                            sc = pscore.tile([128, 2 * QT], F32, tag="sc",
                                             name="sc")
                            emit_scores_pair(sc[:, 0:QT], sc[:, QT:2 * QT], pr,
                                             t, j * QT)
                            sc_t[s][t] = sc
                    # ---- stage B: exp(i-1) ----
                    if 1 <= i <= total:
                        s, t = sweep_of(i - 1)
                        sc = sc_t[s].pop(t)
                        ex = expp.tile([128, 2 * QT], BF16, tag="exp", name="ex")
                        if isinstance(sc, tuple):
                            emit_dve_exp2(ex[:], sc[0][:], sc[1][:])
                            dve_chunks.add((s, t))
                        else:
                            nc.scalar.activation(ex[:], sc[:], EXPF, scale=0.125)
                        ex_t[s][t] = ex
                    # ---- deferred attnv of DVE-exp chunks (the PE stream is
                    # in-order: emitting these at i-2 would stall PE ~2.3us
                    # behind the 3.4us DVE chain; accumulation order is free
                    # thanks to zeroing+start=False, so emit 4 iters late) ----
                    if i in deferred:
                        for (s2, t2, ex2) in deferred.pop(i):
                            j2, pr2 = sweeps[s2]
                            emit_attnv(acc[s2][0], acc[s2][1], ex2[:], pr2, t2)
                    # ---- stage C: attnv(i-2) ----
                    if i >= 2:
                        s, t = sweep_of(i - 2)
                        j, pr = sweeps[s]
                        ex = ex_t[s].pop(t)
                        ft = FLUSH_T0 if s == 0 else FLUSH_T
                        if t < ft:
                            # held until the previous sweep's paced epilogue
                            # has consumed the acc banks this sweep reuses
                            early[s].append((t, ex))
                        elif t == ft:
                            emit_zero_acc(acc[s][0])
                            emit_zero_acc(acc[s][1])
                            for (t2, ex2) in early[s]:
                                emit_attnv(acc[s][0], acc[s][1], ex2[:], pr, t2)
                            early[s] = []
                            emit_attnv(acc[s][0], acc[s][1], ex[:], pr, t)
                        elif (s, t) in dve_chunks:
                            deferred.setdefault(i + 10, []).append((s, t, ex))
                        else:
                            emit_attnv(acc[s][0], acc[s][1], ex[:], pr, t)
                        if t == TCH - 1:
                            # flush any still-deferred attnvs of this sweep
                            # before its epilogue reads the accumulators
                            for rel in sorted(deferred):
                                keep = []
                                for (s2, t2, ex2) in deferred[rel]:
                                    if s2 == s:
                                        emit_attnv(acc[s][0], acc[s][1],
                                                   ex2[:], pr, t2)
                                    else:
                                        keep.append((s2, t2, ex2))
                                if keep:
                                    deferred[rel] = keep
                                else:
                                    del deferred[rel]
                            if pr == 0:
                                nmT_by_j[j] = {0: None, 1: None}
                            nmT_by_j[j][pr] = new_nmT_set()
                            accA, accB = acc[s]
                            nmT = nmT_by_j[j][pr]
                            for mc_ in range(4):
                                for half_, a_ in ((0, accA), (1, accB)):
                                    tail_ = False
                                    def mk_ep(a_, half_, mc_, nmT=nmT,
                                              tail_=tail_):
                                        def fn():
                                            epilogue_unit(a_, nmT, half_, mc_,
                                                          tail=tail_)
                                        return fn
                                    filler.append(("ep", mk_ep(a_, half_, mc_)))
                            acc[s] = None
                            if pr == 1:
                                filler.extend(wo_micro_ops(j, tail=(s == NS - 1)))
                    if i == 4:
                        filler.extend(wo_micro_ops(0))
                    # ---- stage D: paced epilogue/W_o micro-ops: epilogues
                    # 2/iteration; W_o units only on even iterations so their
                    # psw/copy chain never bunches against the DVE queue ----
                    if filler:
                        if filler[0][0] == "ep":
                            filler.pop(0)[1]()
                            if filler and filler[0][0] == "ep":
                                filler.pop(0)[1]()
                        elif i % 2 == 0:
                            filler.pop(0)[1]()

                for kind, fn in filler:
                    fn()

    nc.compile()
    return nc


_nc = None


def kernel(query, key, value, W_k, W_v, W_o):
    global _nc, _last_results, _last_in_maps
    if _nc is None:
        _nc = _build()

    import ml_dtypes
    query = np.asarray(query, dtype=np.float32)
    key = np.asarray(key, dtype=np.float32)
    value = np.asarray(value, dtype=np.float32)
    W_k = np.asarray(W_k, dtype=np.float32)
    W_v = np.asarray(W_v, dtype=np.float32)
    W_o = np.asarray(W_o, dtype=np.float32)

    keyT = [np.ascontiguousarray(key[b].T).astype(ml_dtypes.bfloat16)
            for b in range(B)]
    valT = [np.ascontiguousarray(value[b].T).astype(ml_dtypes.bfloat16)
            for b in range(B)]
    eye = np.eye(128, dtype=np.float32).astype(ml_dtypes.bfloat16)

    in_maps = []
    for b in range(B):
        for g in range(4):
            c0 = g * C
            wo2 = np.ascontiguousarray(
                W_o[:, c0:c0 + C].T.reshape(2, 128, E).transpose(1, 0, 2)
            ).astype(ml_dtypes.bfloat16)
            in_maps.append({
                "keyT": keyT[b],
                "valT": valT[b],
                "qT": np.ascontiguousarray(query[b][:, c0:c0 + C].T),
                "wkT": np.ascontiguousarray(
                    W_k[c0:c0 + C, :].T).astype(ml_dtypes.bfloat16),
                "wvT": np.ascontiguousarray(
                    W_v[c0:c0 + C, :].T).astype(ml_dtypes.bfloat16),
                "wo2": wo2,
                "eye": eye,
            })

    _last_in_maps = in_maps
    res = run_bass_kernel_spmd(
        _nc, in_maps, core_ids=list(range(8)),
        trace=bool(os.environ.get("BASS_TRACE")))
    _last_results = res

    out = np.zeros((B, NQ, E), dtype=np.float32)
    for b in range(B):
        for g in range(4):
            out[b] += res.results[b * 4 + g]["out"]
    return out
